# revision 1
# baseline (speedup 1.0000x reference)
"""GNN message-passing (GCN-mean) kernel for 8 Trainium2 NeuronCores.

Strategy:
  - Nodes partitioned 8 ways (graph parallel); edges assigned to the core
    owning their dst, sorted by (dst-block, src-subrange, src).
  - Per layer: x is AllGathered (bf16) so each core gathers source rows
    locally with batched dma_gather calls (int16 indices into 8 sub-tables,
    4 SWDGE queues), then scatter-add becomes a one-hot matmul per 128-edge
    chunk accumulating into PSUM per 128-node dst block.
  - Per-block fused epilogue: (agg+x)*inv -> PE transpose -> matmul W ->
    bias+ReLU (ACT accum gives LN mean) -> Square (accum gives var) ->
    normalize.  LayerNorm gamma/beta are folded into the next layer's
    weights on the host.  Degree comes free from a ones-column matmul in
    layer 1.  The final layer fuses the output projection.
"""
import math
import numpy as np

NC = 8
P = 128
EPS = 1e-5

_CACHE = {}
_LAST_EXEC = None


# --------------------------------------------------------------------------
# device program
# --------------------------------------------------------------------------
def _build_program(NB, SHP, S_sub, LAYERS, OUT):
    import concourse.bacc as bacc
    import concourse.bass as bass
    import concourse.mybir as mybir
    import concourse.tile as tile
    from concourse.masks import make_identity

    S8 = S_sub * 8          # int16 idx columns per group
    S_blk = 8 * S_sub       # chunks per block
    f32 = mybir.dt.float32
    bf16 = mybir.dt.bfloat16
    Alu = mybir.AluOpType
    Act = mybir.ActivationFunctionType

    nc = bacc.Bacc("TRN2", target_bir_lowering=False, num_devices=NC,
                   num_swdge_queues=4)

    nodes_own = nc.declare_dram_parameter("nodes_own", [SHP, P], f32, isOutput=False)
    idxs = nc.declare_dram_parameter("idxs", [NB, P, 8 * S8], mybir.dt.int16, isOutput=False)
    dstrel = nc.declare_dram_parameter("dstrel", [NB, P, S_blk], f32, isOutput=False)
    cnts = nc.declare_dram_parameter("cnts", [1, NB * 8], mybir.dt.int32, isOutput=False)
    W_in = nc.declare_dram_parameter("W_in", [P, P], f32, isOutput=False)
    Wl = nc.declare_dram_parameter("Wl", [LAYERS, P, P], f32, isOutput=False)
    Wout = nc.declare_dram_parameter("Wout", [P, OUT], f32, isOutput=False)
    b_in_bc = nc.declare_dram_parameter("b_in_bc", [P, P], f32, isOutput=False)
    bl_bc = nc.declare_dram_parameter("bl_bc", [LAYERS, P, P], f32, isOutput=False)
    bout_bc = nc.declare_dram_parameter("bout_bc", [P, OUT], f32, isOutput=False)
    iota_in = nc.declare_dram_parameter("iota", [P, P], f32, isOutput=False)
    out_own = nc.declare_dram_parameter("out_own", [SHP, OUT], f32, isOutput=True)

    rg = [list(range(NC))]

    with tile.TileContext(nc) as tc:
        with (
            tc.tile_pool(name="const", bufs=1) as cpool,
            tc.tile_pool(name="dram", bufs=1, space="DRAM") as dpool,
            tc.tile_pool(name="work", bufs=3) as wpool,
            tc.tile_pool(name="ohp", bufs=8) as ohpool,
            tc.tile_pool(name="xgp", bufs=2) as xgpool,
            tc.tile_pool(name="psum", bufs=2, space="PSUM") as ppool,
        ):
            iota_t = cpool.tile([P, P], f32)
            nc.sync.dma_start(out=iota_t[:], in_=iota_in[:])
            ident = cpool.tile([P, P], f32)
            make_identity(nc, ident[:])
            ones_bf = cpool.tile([P, 1], bf16)
            nc.vector.memset(ones_bf[:], 1.0)
            eps_t = cpool.tile([P, 1], f32)
            nc.vector.memset(eps_t[:], EPS)
            W_in_t = cpool.tile([P, P], f32)
            nc.sync.dma_start(out=W_in_t[:], in_=W_in[:])
            Wout_t = cpool.tile([P, OUT], f32)
            nc.sync.dma_start(out=Wout_t[:], in_=Wout[:])
            bin_t = cpool.tile([P, P], f32)
            nc.sync.dma_start(out=bin_t[:], in_=b_in_bc[:])
            bl_t = []
            for l in range(LAYERS):
                t = cpool.tile([P, P], f32, name=f"bl{l}")
                nc.sync.dma_start(out=t[:], in_=bl_bc[l])
                bl_t.append(t)
            Wl_ts = []
            for l in range(LAYERS):
                t = cpool.tile([P, P], f32, name=f"wl{l}")
                nc.sync.dma_start(out=t[:], in_=Wl[l])
                Wl_ts.append(t)
            bout_t = cpool.tile([P, OUT], f32)
            nc.sync.dma_start(out=bout_t[:], in_=bout_bc[:])
            cnt_t = cpool.tile([1, NB * 8], mybir.dt.int32)
            nc.sync.dma_start(out=cnt_t[:], in_=cnts[:])
            inv_t = cpool.tile([P, NB], f32)

            ag_in = [dpool.tile([SHP, P], bf16, name=f"ag_in{l}") for l in range(LAYERS)]
            x_full = [
                dpool.tile([NC * SHP, P], bf16, addr_space="Shared", name=f"x_full{l}")
                for l in range(LAYERS)
            ]

            # ---------------- Phase A: x0 = nodes @ W_in + b_in ----------
            for b in range(NB):
                nb_t = wpool.tile([P, P], f32, tag="nb")
                nc.sync.dma_start(out=nb_t[:], in_=nodes_own[b * P:(b + 1) * P, :])
                pT = ppool.tile([P, P], f32, tag="mT", space="PSUM")
                nc.tensor.transpose(pT[:], nb_t[:], ident[:])
                nT = wpool.tile([P, P], f32, tag="mTs")
                nc.scalar.copy(out=nT[:], in_=pT[:])
                ph = ppool.tile([P, P], f32, tag="h", space="PSUM")
                nc.tensor.matmul(ph[:], lhsT=nT[:], rhs=W_in_t[:], start=True, stop=True)
                x0b = wpool.tile([P, P], bf16, tag="xnext")
                nc.vector.tensor_tensor(out=x0b[:], in0=ph[:], in1=bin_t[:], op=Alu.add)
                nc.sync.dma_start(out=ag_in[0][b * P:(b + 1) * P, :], in_=x0b[:])
            nc.gpsimd.collective_compute(
                "AllGather", Alu.bypass, replica_groups=rg,
                ins=[ag_in[0][:].opt()], outs=[x_full[0][:].opt()],
            )

            # ---------------- Layers ------------------------------------
            for l in range(LAYERS):
                xf = x_full[l]
                last = l == LAYERS - 1
                for b in range(NB):
                    idx_t = wpool.tile([P, 8 * S8], mybir.dt.int16, tag="idx")
                    nc.sync.dma_start(out=idx_t[:], in_=idxs[b])
                    dst_t = wpool.tile([P, S_blk], f32, tag="dst")
                    nc.sync.dma_start(out=dst_t[:], in_=dstrel[b])
                    xg = xgpool.tile([P, S_blk, P], bf16, tag="xg")
                    if l == 0 and b < 2:
                        # first use of each xg buffer: clear raw SBUF so
                        # never-gathered slots are finite (0 x onehot-0 = 0)
                        nc.vector.memset(xg[:], 0.0)
                    for g in range(8):
                        reg = nc.gpsimd.alloc_register()
                        k = b * 8 + g
                        nc.gpsimd.reg_load(reg, cnt_t[0:1, k:k + 1])
                        nc.gpsimd.dma_gather(
                            out_ap=xg[:, g * S_sub:(g + 1) * S_sub, :],
                            in_ap=xf[g * SHP:(g + 1) * SHP, :],
                            idxs_ap=idx_t[:, g * S8:(g + 1) * S8],
                            num_idxs=S_sub * P,
                            num_idxs_reg=reg,
                            elem_size=P,
                            single_packet=False,
                            queue_num=g % 4,
                        )
                    pagg = ppool.tile([P, P], f32, tag="agg", space="PSUM")
                    if l == 0:
                        pdeg = ppool.tile([P, 1], f32, tag="deg", space="PSUM")
                    for s in range(S_blk):
                        oh = ohpool.tile([P, P], bf16, tag="oh")
                        nc.vector.tensor_scalar(
                            out=oh[:], in0=iota_t[:],
                            scalar1=dst_t[:, s:s + 1], scalar2=None,
                            op0=Alu.is_equal,
                        )
                        nc.tensor.matmul(
                            pagg[:], lhsT=oh[:], rhs=xg[:, s, :],
                            start=(s == 0), stop=(s == S_blk - 1),
                        )
                        if l == 0:
                            nc.tensor.matmul(
                                pdeg[:], lhsT=oh[:], rhs=ones_bf[:],
                                start=(s == 0), stop=(s == S_blk - 1),
                            )
                    # ---- fused epilogue ----
                    if l == 0:
                        dp1 = wpool.tile([P, 1], f32, tag="dp1")
                        nc.vector.tensor_scalar(
                            out=dp1[:], in0=pdeg[:],
                            scalar1=1.0, scalar2=None, op0=Alu.add,
                        )
                        nc.vector.reciprocal(inv_t[:, b:b + 1], dp1[:])
                    xs_bf = wpool.tile([P, P], bf16, tag="xs")
                    nc.sync.dma_start(out=xs_bf[:], in_=ag_in[l][b * P:(b + 1) * P, :])
                    xs = wpool.tile([P, P], f32, tag="xsf")
                    nc.scalar.copy(out=xs[:], in_=xs_bf[:])
                    m0 = wpool.tile([P, P], f32, tag="m0")
                    nc.vector.tensor_tensor(out=m0[:], in0=pagg[:], in1=xs[:], op=Alu.add)
                    m1 = wpool.tile([P, P], f32, tag="m1")
                    nc.vector.tensor_scalar(
                        out=m1[:], in0=m0[:], scalar1=inv_t[:, b:b + 1],
                        scalar2=None, op0=Alu.mult,
                    )
                    pT = ppool.tile([P, P], f32, tag="mT", space="PSUM")
                    nc.tensor.transpose(pT[:], m1[:], ident[:])
                    mT = wpool.tile([P, P], f32, tag="mTs")
                    nc.scalar.copy(out=mT[:], in_=pT[:])
                    ph = ppool.tile([P, P], f32, tag="h", space="PSUM")
                    nc.tensor.matmul(ph[:], lhsT=mT[:], rhs=Wl_ts[l][:], start=True, stop=True)
                    hb = wpool.tile([P, P], f32, tag="hb")
                    nc.vector.tensor_tensor(out=hb[:], in0=ph[:], in1=bl_t[l][:], op=Alu.add)
                    hr = wpool.tile([P, P], f32, tag="hr")
                    mu_s = wpool.tile([P, 1], f32, tag="mus")
                    nc.scalar.activation(hr[:], hb[:], Act.Relu, accum_out=mu_s[:])
                    h2 = wpool.tile([P, P], f32, tag="h2")
                    s2 = wpool.tile([P, 1], f32, tag="s2")
                    nc.scalar.activation(h2[:], hr[:], Act.Square, accum_out=s2[:])
                    musq = wpool.tile([P, 1], f32, tag="musq")
                    nc.vector.tensor_scalar(
                        out=musq[:], in0=mu_s[:], scalar1=mu_s[:, 0:1],
                        scalar2=1.0 / (P * P), op0=Alu.mult, op1=Alu.mult,
                    )
                    var1 = wpool.tile([P, 1], f32, tag="var1")
                    nc.vector.tensor_scalar(
                        out=var1[:], in0=s2[:], scalar1=1.0 / P,
                        scalar2=None, op0=Alu.mult,
                    )
                    var2 = wpool.tile([P, 1], f32, tag="var2")
                    nc.vector.tensor_tensor(out=var2[:], in0=var1[:], in1=musq[:], op=Alu.subtract)
                    std_t = wpool.tile([P, 1], f32, tag="std")
                    nc.scalar.activation(std_t[:], var2[:], Act.Sqrt, bias=eps_t[:, 0:1])
                    rstd = wpool.tile([P, 1], f32, tag="rstd")
                    nc.vector.reciprocal(rstd[:], std_t[:])
                    mu_t = wpool.tile([P, 1], f32, tag="mu")
                    nc.vector.tensor_scalar(
                        out=mu_t[:], in0=mu_s[:], scalar1=1.0 / P,
                        scalar2=None, op0=Alu.mult,
                    )
                    if not last:
                        y_bf = wpool.tile([P, P], bf16, tag="xnext")
                        nc.vector.tensor_scalar(
                            out=y_bf[:], in0=hr[:], scalar1=mu_t[:, 0:1],
                            scalar2=rstd[:, 0:1], op0=Alu.subtract, op1=Alu.mult,
                        )
                        nc.sync.dma_start(out=ag_in[l + 1][b * P:(b + 1) * P, :], in_=y_bf[:])
                    else:
                        y_f = wpool.tile([P, P], f32, tag="yf")
                        nc.vector.tensor_scalar(
                            out=y_f[:], in0=hr[:], scalar1=mu_t[:, 0:1],
                            scalar2=rstd[:, 0:1], op0=Alu.subtract, op1=Alu.mult,
                        )
                        pyT = ppool.tile([P, P], f32, tag="mT", space="PSUM")
                        nc.tensor.transpose(pyT[:], y_f[:], ident[:])
                        yT = wpool.tile([P, P], f32, tag="mTs")
                        nc.scalar.copy(out=yT[:], in_=pyT[:])
                        po = ppool.tile([P, OUT], f32, tag="h", space="PSUM")
                        nc.tensor.matmul(po[:], lhsT=yT[:], rhs=Wout_t[:], start=True, stop=True)
                        ob = wpool.tile([P, OUT], f32, tag="ob")
                        nc.vector.tensor_tensor(out=ob[:], in0=po[:], in1=bout_t[:], op=Alu.add)
                        nc.sync.dma_start(out=out_own[b * P:(b + 1) * P, :], in_=ob[:])
                if not last:
                    nc.gpsimd.collective_compute(
                        "AllGather", Alu.bypass, replica_groups=rg,
                        ins=[ag_in[l + 1][:].opt()], outs=[x_full[l + 1][:].opt()],
                    )
    nc.compile()
    return nc


# --------------------------------------------------------------------------
# host-side sharding prep
# --------------------------------------------------------------------------
def _prep_edges(src, dst, N, SH, SHP, NB):
    E = src.shape[0]
    src = src.astype(np.int64)
    dst = dst.astype(np.int64)
    core = dst // SH
    dst_loc = dst - core * SH
    blk = dst_loc >> 7
    dst_rel = dst_loc & 127
    grp = src // SH
    src_loc = src - grp * SH
    key = (core * NB + blk) * 8 + grp
    order = np.argsort(key * N + src, kind="stable")
    ks = key[order]
    counts = np.bincount(ks, minlength=NC * NB * 8)
    S_sub = max(1, int(math.ceil(counts.max() / P)))
    CAP = S_sub * P
    S_blk = 8 * S_sub

    starts = np.zeros(NC * NB * 8, np.int64)
    np.cumsum(counts[:-1], out=starts[1:])
    pos = np.arange(E, dtype=np.int64) - starts[ks]

    idx16 = np.full((NC, NB, 8, CAP), -1, np.int16)
    idx16.reshape(-1)[ks * CAP + pos] = src_loc[order].astype(np.int16)
    dstrel = np.full((NC, NB, S_blk * P), -1.0, np.float32)
    slot = (ks % 8) * CAP + pos
    dstrel.reshape(-1)[(ks // 8) * (S_blk * P) + slot] = dst_rel[order].astype(np.float32)

    cnt = counts.reshape(NC, NB, 8).astype(np.int32)
    # zero-count guard: gather at least 16 dummy rows (dstrel already -1)
    zc, zb, zg = np.nonzero(cnt == 0)
    if len(zc):
        cnt[zc, zb, zg] = 16
        idx16[zc, zb, zg, :16] = 0

    # wrap idx to the ucode layout: pos i -> [i%16, i//16], replicated 8x
    S8 = CAP // 16
    A = idx16.reshape(NC, NB, 8, S8, 16)
    Bm = A.transpose(0, 1, 2, 4, 3)                      # [NC,NB,8,16,S8]
    C = np.broadcast_to(Bm[:, :, :, None, :, :], (NC, NB, 8, 8, 16, S8))
    idx_dev = np.ascontiguousarray(
        C.transpose(0, 1, 3, 4, 2, 5).reshape(NC, NB, P, 8 * S8)
    )
    dst_dev = np.ascontiguousarray(
        dstrel.reshape(NC, NB, S_blk, P).transpose(0, 1, 3, 2)
    )
    cnt_dev = cnt.reshape(NC, 1, NB * 8)
    return idx_dev, dst_dev, cnt_dev, S_sub


def _run(nc_prog, in_maps):
    import jax
    import numpy as np
    from jax.sharding import Mesh, PartitionSpec, NamedSharding
    from jax.experimental.shard_map import shard_map
    import concourse.mybir as mybir
    from concourse.bass2jax import _bass_exec_p, install_neuronx_cc_hook, partition_id_tensor

    install_neuronx_cc_hook()
    nc = nc_prog
    partition_name = nc.partition_id_tensor.name if nc.partition_id_tensor else None
    in_names, out_names, out_avals, zero_outs = [], [], [], []
    for alloc in nc.m.functions[0].allocations:
        if not isinstance(alloc, mybir.MemoryLocationSet):
            continue
        name = alloc.memorylocations[0].name
        if alloc.kind == "ExternalInput":
            if name != partition_name:
                in_names.append(name)
        elif alloc.kind == "ExternalOutput":
            out_names.append(name)
            shape = tuple(alloc.tensor_shape)
            dtype = mybir.dt.np(alloc.dtype)
            out_avals.append(jax.core.ShapedArray(shape, dtype))
            zero_outs.append(np.zeros(shape, dtype))
    n_params = len(in_names)
    all_in = list(in_names) + list(out_names)
    if partition_name is not None:
        all_in.append(partition_name)

    def _body(*args):
        operands = list(args)
        if partition_name is not None:
            operands.append(partition_id_tensor())
        outs = _bass_exec_p.bind(
            *operands,
            out_avals=tuple(out_avals),
            in_names=tuple(all_in),
            out_names=tuple(out_names),
            lowering_input_output_aliases=(),
            sim_require_finite=False,
            sim_require_nnan=False,
            nc=nc,
        )
        return tuple(outs)

    devices = jax.devices()[:NC]
    mesh = Mesh(np.asarray(devices), ("core",))
    in_specs = (PartitionSpec("core"),) * (n_params + len(out_names))
    out_specs = (PartitionSpec("core"),) * len(out_names)
    fn = jax.jit(
        shard_map(_body, mesh=mesh, in_specs=in_specs, out_specs=out_specs,
                  check_rep=False),
        keep_unused=True,
    )
    concat_in = [
        np.concatenate([np.asarray(in_maps[c][k]) for c in range(NC)], axis=0)
        for k in in_names
    ]
    concat_zero = [np.zeros((NC * z.shape[0], *z.shape[1:]), z.dtype) for z in zero_outs]
    sharding = NamedSharding(mesh, PartitionSpec("core"))
    dev_in = [jax.device_put(a, sharding) for a in concat_in + concat_zero]
    outs = fn(*dev_in)
    jax.block_until_ready(outs)
    res = [
        {name: np.asarray(outs[i]).reshape(NC, *out_avals[i].shape)[c]
         for i, name in enumerate(out_names)}
        for c in range(NC)
    ]
    return res, (fn, dev_in, out_names, out_avals)


def _make_in_maps(inputs, N, SH, SHP, NB, LAYERS, OUT):
    nodes = np.asarray(inputs["nodes"], np.float32)
    src = np.asarray(inputs["src"])
    dst = np.asarray(inputs["dst"])
    W_in = np.asarray(inputs["W_in"], np.float32)
    b_in = np.asarray(inputs["b_in"], np.float32)
    Ws = np.asarray(inputs["Ws"], np.float32)
    bs = np.asarray(inputs["bs"], np.float32)
    gammas = np.asarray(inputs["gammas"], np.float32)
    betas = np.asarray(inputs["betas"], np.float32)
    W_out = np.asarray(inputs["W_out"], np.float32)
    b_out = np.asarray(inputs["b_out"], np.float32)

    idx_dev, dst_dev, cnt_dev, S_sub = _prep_edges(src, dst, N, SH, SHP, NB)

    # fold LayerNorm gamma/beta into the following layer's weights
    Wl = np.zeros((LAYERS, P, P), np.float32)
    bl = np.zeros((LAYERS, P), np.float32)
    Wl[0] = Ws[0]
    bl[0] = bs[0]
    for l in range(1, LAYERS):
        Wl[l] = gammas[l - 1][:, None] * Ws[l]
        bl[l] = betas[l - 1] @ Ws[l] + bs[l]
    Wout = gammas[LAYERS - 1][:, None] * W_out
    bout = betas[LAYERS - 1] @ W_out + b_out

    iota = np.tile(np.arange(P, dtype=np.float32), (P, 1))
    b_in_bc = np.tile(b_in, (P, 1)).astype(np.float32)
    bl_bc = np.stack([np.tile(bl[l], (P, 1)) for l in range(LAYERS)])
    bout_bc = np.tile(bout, (P, 1)).astype(np.float32)

    in_maps = []
    for c in range(NC):
        nsh = np.zeros((SHP, P), np.float32)
        nsh[:SH] = nodes[c * SH:(c + 1) * SH]
        in_maps.append({
            "nodes_own": nsh,
            "idxs": idx_dev[c],
            "dstrel": dst_dev[c],
            "cnts": cnt_dev[c],
            "W_in": W_in,
            "Wl": Wl,
            "Wout": Wout,
            "b_in_bc": b_in_bc,
            "bl_bc": bl_bc,
            "bout_bc": bout_bc,
            "iota": iota,
        })
    return in_maps, S_sub


def kernel(**inputs):
    nodes = np.asarray(inputs["nodes"])
    N = nodes.shape[0]
    LAYERS = np.asarray(inputs["Ws"]).shape[0]
    OUT = np.asarray(inputs["W_out"]).shape[1]
    assert N % NC == 0
    SH = N // NC
    SHP = (SH + P - 1) // P * P
    NB = SHP // P
    assert SHP <= 32767, "int16 gather index limit"

    in_maps, S_sub = _make_in_maps(inputs, N, SH, SHP, NB, LAYERS, OUT)

    key = (NB, SHP, S_sub, LAYERS, OUT)
    if key not in _CACHE:
        _CACHE[key] = _build_program(NB, SHP, S_sub, LAYERS, OUT)
    nc_prog = _CACHE[key]

    res, exec_info = _run(nc_prog, in_maps)
    global _LAST_EXEC
    _LAST_EXEC = exec_info
    out = np.concatenate([res[c]["out_own"][:SH] for c in range(NC)], axis=0)
    return out.astype(np.float32)

